# revision 15
# baseline (speedup 1.0000x reference)
"""Cross-attention kernel for Trainium2, 8-core SPMD.

Sharding: core = b*4 + g (b: batch of 2, g: head-group of 4 heads = 256
q/k/v feature cols). Wq/Wk/Wv column-sharded, Wo row-sharded; the Wo
all-reduce is done host-side when unsharding (sum of partials).

v2 design (vs v1 baseline):
  - x/ctx shipped bf16 (halves HBM traffic); LN stats from bf16.
  - rstd = exp(-0.5*ln(var+eps)) on ACT: stays in the exp activation
    table, so the fused loop never reloads act tables.
  - transposes via DMA xbar (dma_start transpose=True) instead of PE:
    frees ~73k PE cycles and all PSUM->SBUF transpose copies.
  - fused pass B: per 128-row ctx block: LN -> cnT -> kT/v matmuls ->
    scores(hp0, lagged 2 blocks) -> exp -> AV(hp0). Keeps PE dense so
    HAM stays at full clock (v1 ran attention at half clock).
  - pass C: scores/exp/AV for hp1 (ACT-paced).
  - score matmuls K=64: h2=0/1 issued back-to-back at base partitions
    0/64 -> row-tiled concurrent pairs (~2x effective).
  - softmax denominator from a ones-column appended to V (row 64 of
    the AV output); normalization via ln/exp reciprocal + gpsimd
    partition broadcast.
"""

import numpy as np
import ml_dtypes

import concourse.bass as bass
import concourse.tile as tile
from concourse import bacc, mybir
from concourse.bass_utils import run_bass_kernel_spmd

EMB = 1024
TX = 1024
TC = 8192
DL = 256          # per-core q/k/v cols (4 heads x 64)
N_CORES = 8
NBLK = TC // 128  # 64 ctx blocks
NPAIR = NBLK // 2
GLAG = 5          # double-block lag between kv-build and scores


def _patch_act_tables():
    """Prioritize the combined ln+exp activation table so the fused
    loop's Ln (rstd) and Exp (softmax) instructions share one table:
    without this the compiler alternates natural_log/exp_and_others,
    paying a 1.28us ACT_TABLE_LOAD per switch (~190us per run). The
    inserted load ids are positions in the priority list, so remap them
    back to act_info.json indices for walrus."""
    import functools
    import concourse.bacc as bacc_mod
    from concourse import hw_specs

    if getattr(bacc_mod, "_act_tables_patched", False):
        return
    orig_tables = hw_specs.get_activation_tables.__wrapped__

    @functools.cache
    def reordered(arch):
        t = dict(orig_tables(arch))
        key = "natural_log_exp_and_others"
        t2 = {key: t[key]}
        t2.update({k: v for k, v in t.items() if k != key})
        return t2

    bacc_mod.get_activation_tables = reordered
    orig_insert = bacc_mod.Bacc.insert_act_table_loads

    def patched_insert(self):
        orig_insert(self)
        json_order = list(orig_tables(self.m.arch).keys())
        my_order = list(reordered(self.m.arch).keys())
        remap = {i: json_order.index(name) for i, name in enumerate(my_order)}
        for b in self.main_func.blocks:
            for ins in b.instructions:
                if isinstance(ins, mybir.InstLoadActFuncSet):
                    ins.act_func_set_id = remap[ins.act_func_set_id]

    bacc_mod.Bacc.insert_act_table_loads = patched_insert
    bacc_mod._act_tables_patched = True

F32 = mybir.dt.float32
BF16 = mybir.dt.bfloat16
AF = mybir.AluOpType
ACTF = mybir.ActivationFunctionType
PSUM = bass.MemorySpace.PSUM
BF16_NP = ml_dtypes.bfloat16
EPS = 1e-5


def _ln_stats(nc, stat_p, xt, eps_sb):
    """mean + rstd of a [128, 1024] tile (any dtype); rstd via ln/exp."""
    st = stat_p.tile([128, 2, 6], F32)
    nc.vector.bn_stats(out=st[:, 0, :], in_=xt[:, 0:512])
    nc.vector.bn_stats(out=st[:, 1, :], in_=xt[:, 512:1024])
    mv = stat_p.tile([128, 2], F32)
    nc.vector.bn_aggr(out=mv, in_=st)
    lnv = stat_p.tile([128, 1], F32)
    nc.scalar.activation(out=lnv, in_=mv[:, 1:2], func=ACTF.Ln, bias=eps_sb[:, 0:1])
    rstd = stat_p.tile([128, 1], F32)
    nc.scalar.activation(out=rstd, in_=lnv, func=ACTF.Exp, scale=-0.5)
    return mv, rstd


def build_nc():
    from contextlib import ExitStack

    _patch_act_tables()
    nc = bacc.Bacc("TRN2", target_bir_lowering=False, debug=False,
                   num_devices=N_CORES)

    x_d = nc.dram_tensor("x", [TX, EMB], BF16, kind="ExternalInput")
    ctx_d = nc.dram_tensor("ctx", [TC, EMB], BF16, kind="ExternalInput")
    wq_d = nc.dram_tensor("wq", [128, 8, DL], BF16, kind="ExternalInput")
    wk_d = nc.dram_tensor("wk", [128, 8, DL], BF16, kind="ExternalInput")
    wv_d = nc.dram_tensor("wv", [128, 8, DL], BF16, kind="ExternalInput")
    wo_d = nc.dram_tensor("wo", [128, 2, EMB], BF16, kind="ExternalInput")
    cq_d = nc.dram_tensor("cq", [128, 2], F32, kind="ExternalInput")
    ck_d = nc.dram_tensor("ck", [128, 2], F32, kind="ExternalInput")
    cv_d = nc.dram_tensor("cv", [128, DL], F32, kind="ExternalInput")
    y_d = nc.dram_tensor("y", [TX, EMB], F32, kind="ExternalOutput")

    with tile.TileContext(nc) as tc, ExitStack() as top:
        consts = top.enter_context(tc.tile_pool(name="consts", bufs=1))
        wq_sb = consts.tile([128, 8, DL], BF16)
        nc.sync.dma_start(out=wq_sb, in_=wq_d[:])
        wk_sb = consts.tile([128, 8, DL], BF16)
        nc.sync.dma_start(out=wk_sb, in_=wk_d[:])
        wv_sb = consts.tile([128, 8, DL], BF16)
        nc.sync.dma_start(out=wv_sb, in_=wv_d[:])
        wo_sb = consts.tile([128, 2, EMB], BF16)
        nc.sync.dma_start(out=wo_sb, in_=wo_d[:])
        cq_sb = consts.tile([128, 2], F32)
        nc.sync.dma_start(out=cq_sb, in_=cq_d[:])
        ck_sb = consts.tile([128, 2], F32)
        nc.sync.dma_start(out=ck_sb, in_=ck_d[:])
        cv_sb = consts.tile([128, DL], F32)
        nc.sync.dma_start(out=cv_sb, in_=cv_d[:])
        eps_sb = consts.tile([128, 1], F32)
        nc.vector.memset(eps_sb[:], EPS)

        QT_sb = consts.tile([128, 2, TX], BF16)     # [2h2*64, dch, q]

        # ---- long-lived K/V ----
        kv_pool = top.enter_context(tc.tile_pool(name="kv", bufs=1))
        kT = [kv_pool.tile([128, TC], BF16, name=f"kT{i}") for i in range(2)]
        v_sb = kv_pool.tile([128, NBLK, 4, 65], BF16)
        nc.vector.memset(v_sb[:, :, :, 64:65], 1.0)

        outT_sb = kv_pool.tile([128, 2, TX], BF16)

        # ---- pass B: ctx -> kT/v fused with attention hp=0 ----
        att_state = {}

        def scores_exp(nc, hp, ac, qh, sp_pool, pt_pool):
            """Paired (row-tiled h2=0/1) score matmuls + exp for one
            (block, query-half)."""
            sp = sp_pool.tile([128, 2, 512], F32)
            for h2 in range(2):
                nc.tensor.matmul(
                    sp[:, h2, :],
                    kT[hp][h2 * 64:(h2 + 1) * 64, ac * 128:(ac + 1) * 128],
                    QT_sb[h2 * 64:(h2 + 1) * 64, hp, qh * 512:(qh + 1) * 512],
                    start=True, stop=True,
                )
            pt = pt_pool.tile([128, 2, 512], BF16)
            nc.scalar.activation(
                out=pt[:].rearrange("p a b -> p (a b)"),
                in_=sp[:].rearrange("p a b -> p (a b)"),
                func=ACTF.Exp)
            att_state[(hp, ac, qh)] = pt

        def av(nc, hp, ac, qh, oT):
            pt = att_state.pop((hp, ac, qh))
            for h2 in range(2):
                nc.tensor.matmul(
                    oT[h2][0:65, qh * 512:(qh + 1) * 512],
                    v_sb[:, ac, hp * 2 + h2, :],
                    pt[:, h2, :],
                    start=(ac == 0), stop=(ac == NBLK - 1),
                )

        def epilogue_hp(nc, hp, oT, den_p):
            for h2 in range(2):
                lnd = den_p.tile([1, TX], F32)
                nc.scalar.activation(out=lnd, in_=oT[h2][64:65, :], func=ACTF.Ln)
                rec = den_p.tile([1, TX], F32)
                nc.scalar.activation(out=rec, in_=lnd, func=ACTF.Exp, scale=-1.0)
                rrep = den_p.tile([64, TX], F32)
                nc.gpsimd.partition_broadcast(rrep[:], rec[0:1, :])
                nc.vector.tensor_mul(
                    out=outT_sb[h2 * 64:(h2 + 1) * 64, hp, :],
                    in0=oT[h2][0:64, :], in1=rrep,
                )

        with ExitStack() as pb:
            cpool = pb.enter_context(tc.tile_pool(name="cp", bufs=7))
            zpool = pb.enter_context(tc.tile_pool(name="zp", bufs=7))
            stat2 = pb.enter_context(tc.tile_pool(name="st2", bufs=4))
            pmv_p = pb.enter_context(tc.tile_pool(name="pmv", bufs=4))
            cnT_p = pb.enter_context(tc.tile_pool(name="cnT", bufs=4))
            xnT_p = pb.enter_context(tc.tile_pool(name="xnT", bufs=1))
            kp_ps = pb.enter_context(tc.tile_pool(name="kpps", bufs=1, space=PSUM))
            vp_ps = pb.enter_context(tc.tile_pool(name="vpps", bufs=1, space=PSUM))
            sp_ps0 = pb.enter_context(tc.tile_pool(name="sp0", bufs=1, space=PSUM))
            ot_ps0 = pb.enter_context(tc.tile_pool(name="ot0", bufs=1, space=PSUM))
            pt_p0 = pb.enter_context(tc.tile_pool(name="ptp0", bufs=8))
            den_p0 = pb.enter_context(tc.tile_pool(name="den0", bufs=1))

            oT0 = [ot_ps0.tile([128, TX], F32, name=f"oT0_{i}") for i in range(2)]
            xnT = xnT_p.tile([128, 8, TX], BF16)

            def pair_ln(nc, pool_src, dst_zs, g):
                """stats + rstd (ln/exp) + normalize for blocks 2g, 2g+1
                of a [*, 1024] bf16 source already loaded into tiles."""
                pmv = pmv_p.tile([128, 2, 2], F32)
                for j in (0, 1):
                    st = stat2.tile([128, 2, 6], F32)
                    ctj = pool_src[2 * g + j]
                    nc.vector.bn_stats(out=st[:, 0, :], in_=ctj[:, 0:512])
                    nc.vector.bn_stats(out=st[:, 1, :], in_=ctj[:, 512:1024])
                    nc.vector.bn_aggr(out=pmv[:, j, :], in_=st)
                lnv = stat2.tile([128, 2], F32)
                nc.scalar.activation(out=lnv, in_=pmv[:, :, 1],
                                     func=ACTF.Ln, bias=eps_sb[:, 0:1])
                rstd2 = stat2.tile([128, 2], F32)
                nc.scalar.activation(out=rstd2, in_=lnv, func=ACTF.Exp,
                                     scale=-0.5)
                for j in (0, 1):
                    b = 2 * g + j
                    z = zpool.tile([128, EMB], BF16)
                    nc.vector.tensor_scalar(
                        out=z, in0=pool_src.pop(b), scalar1=pmv[:, j, 0:1],
                        scalar2=rstd2[:, j:j + 1],
                        op0=AF.subtract, op1=AF.mult,
                    )
                    dst_zs[b] = z

            # Software pipeline over double-blocks g (256 ctx rows):
            #   load ct-pair(g); transpose z-pair(g-1); LN pair g;
            #   kv matmuls pair g-2; scores+exp pair g-GLAG; AV one pair
            #   behind scores. x/Q prep rides iterations g=0..5.
            cts, zs, cnTs = {}, {}, {}
            xts, xzs = {}, {}
            for g in range(NPAIR + GLAG + 1):
                if g < NPAIR:
                    for j in (0, 1):
                        b = 2 * g + j
                        ct = cpool.tile([128, EMB], BF16)
                        nc.sync.dma_start(out=ct, in_=ctx_d[b * 128:(b + 1) * 128, :])
                        cts[b] = ct
                if g < 4:
                    for j in (0, 1):
                        b = 2 * g + j
                        xt = cpool.tile([128, EMB], BF16)
                        nc.sync.dma_start(out=xt, in_=x_d[b * 128:(b + 1) * 128, :])
                        xts[b] = xt
                # transposes for pair g-1 (z ready since last iteration)
                if 1 <= g <= NPAIR:
                    pT = g - 1
                    cnT = cnT_p.tile([128, 8, 256], BF16)
                    for j in (0, 1):
                        nc.sync.dma_start(
                            out=cnT[:, :, j * 128:(j + 1) * 128],
                            in_=zs.pop(2 * pT + j), transpose=True)
                    cnTs[pT] = cnT
                if 1 <= g <= 4:
                    for j in (0, 1):
                        b = 2 * (g - 1) + j
                        nc.sync.dma_start(
                            out=xnT[:, :, b * 128:(b + 1) * 128],
                            in_=xzs.pop(b), transpose=True)
                if g < NPAIR:
                    pair_ln(nc, cts, zs, g)
                if g < 4:
                    pair_ln(nc, xts, xzs, g)
                if g == 5:
                    # QT matmuls (xnT complete since g=4)
                    for dch in range(2):
                        qsp = sp_ps0.tile([128, 2, 512], F32, name="sp")
                        for qh in range(2):
                            for ec in range(8):
                                nc.tensor.matmul(
                                    qsp[:, qh, :],
                                    wq_sb[:, ec, dch * 128:(dch + 1) * 128],
                                    xnT[:, ec, qh * 512:(qh + 1) * 512],
                                    start=(ec == 0), stop=(ec == 7),
                                )
                        nc.vector.tensor_scalar_add(
                            out=QT_sb[:, dch, :],
                            in0=qsp[:].rearrange("p a b -> p (a b)"),
                            scalar1=cq_sb[:, dch:dch + 1],
                        )

                a0 = 2 * (g - GLAG)
                b0 = a0 - 2
                p = g - 2
                cnTp = cnTs.pop(p) if 0 <= p < NPAIR else None
                if 0 <= a0 < NBLK:
                    scores_exp(nc, 0, a0, 0, sp_ps0, pt_p0)
                kp = vp = None
                if cnTp is not None:
                    kp = kp_ps.tile([128, 2, 256], F32)
                    for dch in range(2):
                        for ec in range(8):
                            nc.tensor.matmul(
                                kp[:, dch, :],
                                wk_sb[:, ec, dch * 128:(dch + 1) * 128],
                                cnTp[:, ec, :],
                                start=(ec == 0), stop=(ec == 7),
                            )
                if 0 <= a0 < NBLK:
                    scores_exp(nc, 0, a0, 1, sp_ps0, pt_p0)
                if cnTp is not None:
                    vp = vp_ps.tile([128, 2, 256], F32)
                    for j in (0, 1):
                        for ec in range(8):
                            nc.tensor.matmul(
                                vp[:, j, :],
                                cnTp[:, ec, j * 128:(j + 1) * 128],
                                wv_sb[:, ec, :],
                                start=(ec == 0), stop=(ec == 7),
                            )
                if 0 <= a0 + 1 < NBLK:
                    scores_exp(nc, 0, a0 + 1, 0, sp_ps0, pt_p0)
                if 0 <= b0:
                    av(nc, 0, b0, 0, oT0)
                    av(nc, 0, b0, 1, oT0)
                if 0 <= a0 + 1 < NBLK:
                    scores_exp(nc, 0, a0 + 1, 1, sp_ps0, pt_p0)
                if 0 <= b0 + 1 < NBLK:
                    av(nc, 0, b0 + 1, 0, oT0)
                    av(nc, 0, b0 + 1, 1, oT0)
                if cnTp is not None:
                    for dch in range(2):
                        nc.vector.tensor_scalar_add(
                            out=kT[dch][:, 2 * p * 128:(2 * p + 2) * 128],
                            in0=kp[:, dch, :], scalar1=ck_sb[:, dch:dch + 1],
                        )
                    for j in (0, 1):
                        nc.vector.tensor_add(
                            out=v_sb[:, 2 * p + j, :, 0:64],
                            in0=vp[:, j, :].rearrange("p (h d) -> p h d", d=64),
                            in1=cv_sb[:].rearrange("p (h d) -> p h d", d=64),
                        )

            epilogue_hp(nc, 0, oT0, den_p0)

        # ---- pass C: attention hp=1 ----
        with ExitStack() as pc:
            sp_ps1 = pc.enter_context(tc.tile_pool(name="sp1", bufs=2, space=PSUM))
            ot_ps1 = pc.enter_context(tc.tile_pool(name="ot1", bufs=1, space=PSUM))
            pt_p1 = pc.enter_context(tc.tile_pool(name="ptp1", bufs=4))
            den_p1 = pc.enter_context(tc.tile_pool(name="den1", bufs=1))

            oT1 = [ot_ps1.tile([128, TX], F32, name=f"oT1_{i}") for i in range(2)]
            for ac in range(NBLK):
                scores_exp(nc, 1, ac, 0, sp_ps1, pt_p1)
                if ac > 0:
                    av(nc, 1, ac - 1, 0, oT1)
                scores_exp(nc, 1, ac, 1, sp_ps1, pt_p1)
                if ac > 0:
                    av(nc, 1, ac - 1, 1, oT1)
            av(nc, 1, NBLK - 1, 0, oT1)
            av(nc, 1, NBLK - 1, 1, oT1)

            epilogue_hp(nc, 1, oT1, den_p1)

        # ---- out proj: y = outT.T @ woP ----
        with ExitStack() as p4:
            y_ps = p4.enter_context(tc.tile_pool(name="yps", bufs=4, space=PSUM))
            y_p = p4.enter_context(tc.tile_pool(name="ysb", bufs=3))
            for qt in range(8):
                ysb = y_p.tile([128, EMB], F32)
                for eh in range(2):
                    ps = y_ps.tile([128, 512], F32)
                    for dch in range(2):
                        nc.tensor.matmul(
                            ps[:],
                            outT_sb[:, dch, qt * 128:(qt + 1) * 128],
                            wo_sb[:, dch, eh * 512:(eh + 1) * 512],
                            start=(dch == 0), stop=(dch == 1),
                        )
                    nc.vector.tensor_copy(out=ysb[:, eh * 512:(eh + 1) * 512], in_=ps[:])
                nc.sync.dma_start(out=y_d[qt * 128:(qt + 1) * 128, :], in_=ysb)

    nc.compile()
    return nc


_NC_CACHE = []


def get_nc():
    if not _NC_CACHE:
        _NC_CACHE.append(build_nc())
    return _NC_CACHE[0]


def make_in_maps(inputs):
    x = np.asarray(inputs["x"], np.float32)
    context = np.asarray(inputs["context"], np.float32)
    Wq = np.asarray(inputs["Wq"], np.float32)
    Wk = np.asarray(inputs["Wk"], np.float32)
    Wv = np.asarray(inputs["Wv"], np.float32)
    Wo = np.asarray(inputs["Wo"], np.float32)
    g1 = np.asarray(inputs["g1"], np.float32)
    b1 = np.asarray(inputs["b1"], np.float32)
    g2 = np.asarray(inputs["g2"], np.float32)
    b2 = np.asarray(inputs["b2"], np.float32)
    scale = 1.0 / np.sqrt(64.0)

    x16 = [np.ascontiguousarray(x[b].astype(BF16_NP)) for b in range(2)]
    c16 = [np.ascontiguousarray(context[b].astype(BF16_NP)) for b in range(2)]

    in_maps = []
    for core in range(N_CORES):
        b, g = core // 4, core % 4
        r = slice(g * DL, (g + 1) * DL)
        wqt = (scale * (g1[:, None] * Wq[r].T)).astype(BF16_NP)   # [1024, 256]
        wkt = (g2[:, None] * Wk[r].T).astype(BF16_NP)
        wvt = (g2[:, None] * Wv[r].T).astype(BF16_NP)
        wop = Wo[:, r].T.astype(BF16_NP)                          # [256, 1024]
        cq = (scale * (b1 @ Wq[r].T)).astype(np.float32)          # [256]
        ck = (b2 @ Wk[r].T).astype(np.float32)
        cv = (b2 @ Wv[r].T).astype(np.float32)
        in_maps.append({
            "x": x16[b],
            "ctx": c16[b],
            "wq": np.ascontiguousarray(wqt.reshape(8, 128, DL).transpose(1, 0, 2)),
            "wk": np.ascontiguousarray(wkt.reshape(8, 128, DL).transpose(1, 0, 2)),
            "wv": np.ascontiguousarray(wvt.reshape(8, 128, DL).transpose(1, 0, 2)),
            "wo": np.ascontiguousarray(wop.reshape(2, 128, EMB).transpose(1, 0, 2)),
            "cq": np.ascontiguousarray(cq.reshape(2, 128).T),
            "ck": np.ascontiguousarray(ck.reshape(2, 128).T),
            "cv": np.ascontiguousarray(np.tile(cv[None, :], (128, 1))),
        })
    return in_maps


def unshard(results, inputs):
    bo = np.asarray(inputs["bo"], np.float32)
    ys = []
    for b in range(2):
        acc = results[b * 4 + 0]["y"].astype(np.float32).copy()
        for g in range(1, 4):
            acc += results[b * 4 + g]["y"]
        ys.append(acc + bo[None, :])
    return np.stack(ys, axis=0).astype(np.float32)


def kernel(**inputs):
    nc = get_nc()
    in_maps = make_in_maps(inputs)
    res = run_bass_kernel_spmd(nc, in_maps, core_ids=list(range(N_CORES)))
    return unshard(res.results, inputs)
